# revision 28
# baseline (speedup 1.0000x reference)
"""DeepSurv loss v3: bucketed decomposition, all-bf16 unit-stride DVE path.

Buckets: bb = int(T*2048) in [0, 2047]; d1 = bb>>5 (64), d2 = bb&31 (32).
[T_j > T_i] = [d1_j > d1_i] + [d1_j == d1_i]*[d2_j > d2_i] (+ dropped
same-bb residual, ~1e-4 loss error). s_i = H[d1_i] + W[d1_i, d2_i].

v3 vs v2.4 (45.3us):
- Grid tensors transposed to [128, {b|c}, t] with the contiguous j-tile
  index LAST, iota operands materialized per-chunk, everything bf16 ->
  every big DVE op qualifies for 2x_1p perf mode (was 1x broadcast APs).
  The scalar-engine Sign/Relu rhs path (45 ACTIVATEs ~16us) is gone.
- Row lookup packed: the four [32,512] psZ matmuls land at partition
  offsets 0/32/64/96 of one PSUM tile; one ZZ multiply (vs 4) against a
  (h, loss, c)-structured oh2big; one block-diagonal matmul reduces all
  four column sums at once into [4, 512].
- No s_bounce DRAM round trip: the final log/weight phase runs in the
  same [4, 512] column layout (P rows and E rows are DMA-loaded in that
  layout up front) and emits per-(half, loss) num/den partials [4, 2];
  the host sums halves and cores.
- Ln activation-table load pulled forward via a dummy Ln so the final
  phase doesn't eat a 1.3us ACT_TABLE_LOAD.
"""

import sys

sys.path.insert(0, "/opt/trn_rl_repo")

import numpy as np

N = 8192
NCORES = 8
R = N // NCORES  # 1024
RT = R // 128  # 8
NT = N // 128  # 64 j-tiles
B = 64  # d1 buckets
C = 32  # d2 grid
CW = C + 2
EPS = 1e-6
CHW = 16  # j-tiles per build chunk
HR = R // 2  # 512

_CACHE = {}


def _ensure_profile_hook():
    import types

    try:
        from antenv import axon_hooks  # noqa: F401

        return
    except ImportError:
        pass
    mod = types.ModuleType("antenv.axon_hooks")
    mod._hook = None

    def set_axon_ntff_profile_hook(hook):
        mod._hook = hook

    def get_axon_ntff_profile_hook():
        if mod._hook is None:
            try:
                from trn_agent_boot.trn_boot import _ntff_profile_via_ctypes

                mod._hook = _ntff_profile_via_ctypes("/opt/axon/libaxon_pjrt.so")
            except Exception:
                mod._hook = None
        return mod._hook

    mod.set_axon_ntff_profile_hook = set_axon_ntff_profile_hook
    mod.get_axon_ntff_profile_hook = get_axon_ntff_profile_hook
    import antenv

    antenv.axon_hooks = mod
    sys.modules["antenv.axon_hooks"] = mod


def _build():
    import ml_dtypes
    import concourse.bacc as bacc
    import concourse.mybir as mybir
    from concourse.tile import TileContext

    f32 = mybir.dt.float32
    bf16 = mybir.dt.bfloat16
    i32 = mybir.dt.int32
    Alu = mybir.AluOpType
    Act = mybir.ActivationFunctionType

    nc = bacc.Bacc("TRN2")

    all3 = nc.declare_dram_parameter("all3", [3, N], f32, isOutput=False)
    E_all = nc.declare_dram_parameter("E_all", [N], i32, isOutput=False)
    rows3 = nc.declare_dram_parameter("rows3", [3, R], f32, isOutput=False)
    E_rows = nc.declare_dram_parameter("E_rows", [R], i32, isOutput=False)
    out_red = nc.declare_dram_parameter("out_red", [4, 2], f32, isOutput=True)

    dig_bounce = nc.dram_tensor("dig_bounce", [2, R], bf16)

    # iotaFP cols: [0:128) = col index (f32), 128 = partition index,
    # 129 = partition index mod 32
    iotaFP_np = np.concatenate(
        [
            np.arange(128, dtype=np.float32)[None, :].repeat(128, 0),
            np.arange(128, dtype=np.float32)[:, None],
            (np.arange(128, dtype=np.float32) % 32)[:, None],
        ],
        axis=1,
    )
    # value = b, repeated over a CHW-wide t block (same block reused by
    # every chunk since the compare target doesn't depend on t)
    iotaB_np = (
        np.repeat(np.arange(B, dtype=np.float32), CHW)[None, :]
        .repeat(128, 0)
        .astype(ml_dtypes.bfloat16)
    )
    # mask grid col k holds [d2 >= k-1]; both losses derive from it:
    # risk W(b,c) = U(b, c+2), surv W(b,c) = G_s(b) - U(b, c+1)
    iotaC_np = (
        np.repeat(np.arange(CW, dtype=np.float32) - 1.0, CHW)[None, :]
        .repeat(128, 0)
        .astype(ml_dtypes.bfloat16)
    )
    # column-sum folding matmul: ZZ partition block b = p//32 is
    # (h, loss) = (b>>1, b&1); route to output row m = 2*loss + h.
    # Surv blocks get -1: their W' column is U - H' = -s, so the fold
    # restores the sign.
    perm = np.array([0, 2, 1, 3])
    sign = np.array([1.0, -1.0, 1.0, -1.0], dtype=np.float32)
    bd_np = (
        (perm[np.arange(128) // 32][:, None] == np.arange(4)[None, :])
        * sign[np.arange(128) // 32][:, None]
    ).astype(np.float32).astype(ml_dtypes.bfloat16)
    iotaFP_d = nc.inline_tensor(iotaFP_np, name="iotaFP")
    iotaB_d = nc.inline_tensor(iotaB_np, name="iotaB")
    iotaC_d = nc.inline_tensor(iotaC_np, name="iotaC")
    bd_d = nc.inline_tensor(bd_np, name="bdiag")

    with TileContext(nc) as tc:
        with (
            tc.tile_pool(name="const", bufs=1) as cpool,
            tc.tile_pool(name="small", bufs=2) as spool,
            tc.tile_pool(name="psgw", bufs=1, space="PSUM") as psgw_pool,
            tc.tile_pool(name="psh", bufs=1, space="PSUM") as psh_pool,
            tc.tile_pool(name="psz", bufs=1, space="PSUM") as psz_pool,
            tc.tile_pool(name="pss", bufs=1, space="PSUM") as pss_pool,
        ):
            # ---- input + const DMAs, spread across engine queues ----
            rall = cpool.tile([128, 3, RT], f32)
            nc.sync.dma_start(
                out=rall[:], in_=rows3[:].rearrange("k (p t) -> p k t", p=128)
            )
            jall = cpool.tile([128, 3, NT], f32)
            nc.gpsimd.dma_start(
                out=jall[:], in_=all3[:].rearrange("k (p t) -> p k t", p=128)
            )
            Ej_i = cpool.tile([128, NT], i32)
            nc.gpsimd.dma_start(
                out=Ej_i[:], in_=E_all[:].rearrange("(p t) -> p t", p=128)
            )
            iotaB = cpool.tile([128, B, CHW], bf16)
            nc.gpsimd.dma_start(
                out=iotaB[:], in_=iotaB_d[:].rearrange("p (b t) -> p b t", b=B)
            )
            iotaC = cpool.tile([128, CW, CHW], bf16)
            nc.gpsimd.dma_start(
                out=iotaC[:], in_=iotaC_d[:].rearrange("p (c t) -> p c t", c=CW)
            )
            iotaFP = cpool.tile([128, 130], f32)
            nc.sync.dma_start(out=iotaFP[:], in_=iotaFP_d[:])
            iotaF = iotaFP[:, 0:128]
            iotaP = iotaFP[:, 128:129]
            c_col = iotaFP[:, 129:130]

            eps_col = cpool.tile([128, 1], f32)
            nc.vector.memset(eps_col[:], EPS)

            # ---- row digits first: bounce launches ASAP ----
            T_pt = rall[:, 0, :]
            x2r = spool.tile([128, RT], f32, tag="x2r")
            nc.vector.tensor_scalar(x2r[:], T_pt[:], 2048.0, 2047.0, Alu.mult, Alu.min)
            bbr = spool.tile([128, RT], i32, tag="bbr")
            nc.vector.tensor_copy(bbr[:], x2r[:])
            b1r = spool.tile([128, RT], i32, tag="b1r")
            nc.vector.tensor_scalar(b1r[:], bbr[:], 5, None, Alu.arith_shift_right)
            b2r = spool.tile([128, RT], i32, tag="b2r")
            nc.vector.tensor_scalar(b2r[:], bbr[:], 31, None, Alu.bitwise_and)
            dph = spool.tile([128, 2, RT], bf16, tag="dph")
            nc.vector.tensor_copy(dph[:, 0, :], b1r[:])
            nc.vector.tensor_copy(dph[:, 1, :], b2r[:])
            nc.sync.dma_start(
                out=dig_bounce[:].rearrange("k (p r) -> p k r", p=128), in_=dph[:]
            )
            # broadcast reads: d1 digits to 64 partitions (b-compare), d2
            # digits to (h, loss, c)-structured 128 partitions
            d1b = cpool.tile([B, R], bf16)
            nc.gpsimd.dma_start(
                out=d1b[:],
                in_=dig_bounce[0].rearrange("(a r) -> a r", a=1).to_broadcast([B, R]),
            )
            d2big = cpool.tile([128, HR], bf16)
            nc.sync.dma_start(
                out=d2big[0:64, :],
                in_=dig_bounce[1, 0:HR]
                .rearrange("(a r) -> a r", a=1)
                .to_broadcast([64, HR]),
            )
            nc.sync.dma_start(
                out=d2big[64:128, :],
                in_=dig_bounce[1, HR:R]
                .rearrange("(a r) -> a r", a=1)
                .to_broadcast([64, HR]),
            )

            # ---- j-side values + digits ----
            v_r = cpool.tile([128, NT], bf16)
            nc.scalar.activation(v_r[:], jall[:, 1, :], Act.Exp)
            v_sx = spool.tile([128, NT], bf16, tag="v_sx")
            nc.scalar.activation(v_sx[:], jall[:, 2, :], Act.Exp)
            # pull the Ln table load forward, off the critical tail; input
            # depends on v_sx so the scheduler can't hoist it before the Exps
            dumL = spool.tile([1, 1], f32, tag="dumL")
            nc.scalar.activation(dumL[:], v_sx[0:1, 0:1], Act.Ln, bias=1.0)
            # late-needed loads on the sync queue (keep scalar Exp/Ln-only)
            bdiag = cpool.tile([128, 4], bf16)
            nc.sync.dma_start(out=bdiag[:], in_=bd_d[:])
            # final-phase row-major loads: [4, 512] with partition
            # m = 2*loss + h (risk halves at rows 0:2, surv at 2:4)
            P4f = cpool.tile([4, HR], f32)
            nc.sync.dma_start(
                out=P4f[:], in_=rows3[1:3, :].rearrange("k (h r) -> (k h) r", h=2)
            )
            E2 = cpool.tile([2, HR], i32)
            nc.sync.dma_start(
                out=E2[:], in_=E_rows[:].rearrange("(h r) -> h r", h=2)
            )
            x2j = spool.tile([128, NT], f32, tag="x2j")
            nc.vector.tensor_scalar(
                x2j[:], jall[:, 0, :], 2048.0, 2047.0, Alu.mult, Alu.min
            )
            bbj = spool.tile([128, NT], i32, tag="bbj")
            nc.vector.tensor_copy(bbj[:], x2j[:])
            b1j = spool.tile([128, NT], i32, tag="b1j")
            nc.vector.tensor_scalar(b1j[:], bbj[:], 5, None, Alu.arith_shift_right)
            d1j = cpool.tile([128, NT], bf16)
            nc.vector.tensor_copy(d1j[:], b1j[:])
            b2j = spool.tile([128, NT], i32, tag="b2j")
            nc.vector.tensor_scalar(b2j[:], bbj[:], 31, None, Alu.bitwise_and)
            d2j = cpool.tile([128, NT], bf16)
            nc.vector.tensor_copy(d2j[:], b2j[:])

            Ej_f = spool.tile([128, NT], bf16, tag="Ej_f")
            nc.vector.tensor_copy(Ej_f[:], Ej_i[:])
            v_s = cpool.tile([128, NT], bf16)
            nc.vector.tensor_mul(v_s[:], v_sx[:], Ej_f[:])

            # ---- grid builds (all bf16, unit-stride last dim = 2x DVE)
            # interleaved with G/W accumulation on the PE ----
            oh1T = cpool.tile([128, B, NT], bf16)
            rhsT = cpool.tile([128, 2, CW, NT], bf16)
            psGW = psgw_pool.tile([B, 2 * CW], f32)
            # shrinking chunks: the PE (slower per tile) catches up on the
            # small final chunks instead of trailing a full 16-tile chunk
            bounds = [0, 8, 24, 40, 56, 64]
            for lo, hi in zip(bounds[:-1], bounds[1:]):
                w = hi - lo
                nc.vector.tensor_tensor(
                    oh1T[:, :, lo:hi],
                    d1j[:, lo:hi].unsqueeze(1).broadcast_to([128, B, w]),
                    iotaB[:, :, 0:w],
                    Alu.is_equal,
                )
                # one mask grid serves both losses: col k = [d2 >= k-1]
                mk = spool.tile([128, CW, CHW], bf16, tag="mk")
                nc.vector.tensor_tensor(
                    mk[:, :, 0:w],
                    d2j[:, lo:hi].unsqueeze(1).broadcast_to([128, CW, w]),
                    iotaC[:, :, 0:w],
                    Alu.is_ge,
                )
                nc.vector.tensor_tensor(
                    rhsT[:, 0, :, lo:hi],
                    mk[:, :, 0:w],
                    v_r[:, lo:hi].unsqueeze(1).broadcast_to([128, CW, w]),
                    Alu.mult,
                )
                nc.vector.tensor_tensor(
                    rhsT[:, 1, :, lo:hi],
                    mk[:, :, 0:w],
                    v_s[:, lo:hi].unsqueeze(1).broadcast_to([128, CW, w]),
                    Alu.mult,
                )
                for t in range(lo, hi):
                    nc.tensor.matmul(
                        psGW[:],
                        lhsT=oh1T[:, :, t],
                        rhs=rhsT[:, :, :, t],
                        start=(t == 0),
                        stop=(t == NT - 1),
                    )

            # UT masks for the H fold (needed only at fold time)
            UTg = cpool.tile([B, B], bf16)
            nc.vector.tensor_scalar(
                UTg[:], iotaF[0:B, 0:B], iotaP[0:B, :], None, Alu.is_lt
            )
            # inclusive prefix for surv: H'_s(b) = sum_{b' <= b} G_s(b')
            UTl = cpool.tile([B, B], bf16)
            nc.vector.tensor_scalar(
                UTl[:], iotaF[0:B, 0:B], iotaP[0:B, :], None, Alu.is_ge
            )

            # ---- row-side onehots (ready while G/W accumulates) ----
            oh1_i = cpool.tile([B, R], bf16)
            nc.vector.tensor_scalar(oh1_i[:], d1b[:], iotaP[0:B, :], None, Alu.is_equal)
            oh2big = cpool.tile([128, HR], bf16)
            nc.vector.tensor_scalar(oh2big[:], d2big[:], c_col, None, Alu.is_equal)

            # final-phase operand prep (off critical path)
            P4 = cpool.tile([4, HR], bf16)
            nc.vector.tensor_copy(P4[:], P4f[:])
            Ef2 = cpool.tile([2, HR], bf16)
            nc.vector.tensor_copy(Ef2[:], E2[:])

            # ---- H fold: G columns -> prefix sums -> fold into W' ----
            Gsb = spool.tile([B, 2], bf16, tag="Gsb")
            nc.vector.tensor_copy(Gsb[:, 0:1], psGW[:, 0:1])
            nc.vector.tensor_copy(Gsb[:, 1:2], psGW[:, CW : CW + 1])
            psH = psh_pool.tile([B, 2], f32)
            nc.tensor.matmul(
                psH[:, 0:1], lhsT=UTg[:], rhs=Gsb[:, 0:1], start=True, stop=True
            )
            nc.tensor.matmul(
                psH[:, 1:2], lhsT=UTl[:], rhs=Gsb[:, 1:2], start=True, stop=True
            )
            Hsb = spool.tile([B, 2], f32, tag="Hsb")
            nc.vector.tensor_copy(Hsb[:], psH[:])
            # W_pair columns: [0:32) = risk W' = U_r(:, c+2) + H_r,
            # [32:64) = surv U_s(:, c+1) - H'_s = -s_surv (bdiag flips it)
            W_pair = cpool.tile([B, 2 * C], bf16)
            nc.vector.tensor_scalar(
                W_pair[:, 0:C], psGW[:, 2:CW], Hsb[:, 0:1], None, Alu.add
            )
            nc.vector.tensor_scalar(
                W_pair[:, C : 2 * C],
                psGW[:, CW + 1 : CW + 1 + C],
                Hsb[:, 1:2],
                None,
                Alu.subtract,
            )

            # ---- packed row lookup ----
            # psZbig partitions 32m:32m+32, m = 2h + loss; one matmul per
            # half covers both losses via the paired lhsT
            psZbig = psz_pool.tile([128, HR], f32)
            for h in range(2):
                nc.tensor.matmul(
                    psZbig[64 * h : 64 * h + 64, :],
                    lhsT=W_pair[:],
                    rhs=oh1_i[:, h * HR : (h + 1) * HR],
                    start=True,
                    stop=True,
                )
            ZZ = spool.tile([128, HR], bf16, tag="ZZ")
            nc.vector.tensor_mul(ZZ[0:64, :], psZbig[0:64, :], oh2big[0:64, :])
            nc.vector.tensor_mul(ZZ[64:128, :], psZbig[64:128, :], oh2big[64:128, :])
            psS4 = pss_pool.tile([4, HR], f32)
            nc.tensor.matmul(psS4[:], lhsT=bdiag[:], rhs=ZZ[:], start=True, stop=True)

            # ---- final phase in [4, 512] column layout ----
            e_eff = spool.tile([4, HR], bf16, tag="e_eff")
            nc.vector.tensor_scalar(e_eff[:], psS4[:], 0.0, None, Alu.is_gt)
            nc.vector.tensor_mul(e_eff[0:2, :], e_eff[0:2, :], Ef2[:])
            red = spool.tile([4, 2], f32, tag="red")
            nc.vector.tensor_reduce(
                red[:, 1:2], e_eff[:], axis=mybir.AxisListType.X, op=Alu.add
            )
            # clamp: surv s = H'-U cancels in bf16 and can round slightly
            # negative on empty rows; Ln(neg) would NaN through the e_eff
            # gate (NaN*0=NaN)
            srl = spool.tile([4, HR], f32, tag="srl")
            nc.vector.tensor_scalar(srl[:], psS4[:], EPS, None, Alu.max)
            lg = spool.tile([4, HR], bf16, tag="lg")
            nc.scalar.activation(lg[:], srl[:], Act.Ln)
            wv = spool.tile([4, HR], bf16, tag="wv")
            nc.vector.tensor_sub(wv[:], P4[:], lg[:])
            nc.vector.tensor_mul(wv[:], wv[:], e_eff[:])
            nc.vector.tensor_reduce(
                red[:, 0:1], wv[:], axis=mybir.AxisListType.X, op=Alu.add
            )
            nc.sync.dma_start(out=out_red[:], in_=red[:])

    nc.finalize()
    return nc


def _get_nc():
    if "nc" not in _CACHE:
        _CACHE["nc"] = _build()
    return _CACHE["nc"]


def make_in_maps(P_risk, P_surv, T, E):
    T = np.ascontiguousarray(np.asarray(T, dtype=np.float32))
    P_risk = np.ascontiguousarray(np.asarray(P_risk, dtype=np.float32))
    P_surv = np.ascontiguousarray(np.asarray(P_surv, dtype=np.float32))
    E = np.ascontiguousarray(np.asarray(E, dtype=np.int32))
    all3 = np.ascontiguousarray(np.stack([T, P_risk, P_surv], axis=0))
    in_maps = []
    for c in range(NCORES):
        sl = slice(c * R, (c + 1) * R)
        in_maps.append(
            {
                "all3": all3,
                "E_all": E,
                "rows3": np.ascontiguousarray(all3[:, sl]),
                "E_rows": np.ascontiguousarray(E[sl]),
            }
        )
    return in_maps


def combine_partials(parts):
    # parts: [4, 2] per core; partition m = 2*loss + h, col 0 = num, 1 = den
    acc = np.zeros((4, 2), dtype=np.float64)
    for p in parts:
        acc += np.asarray(p, dtype=np.float64)
    num_r = acc[0, 0] + acc[1, 0]
    den_r = acc[0, 1] + acc[1, 1]
    num_s = acc[2, 0] + acc[3, 0]
    den_s = acc[2, 1] + acc[3, 1]
    loss_risk = np.float32(-(num_r / den_r))
    loss_surv = np.float32(-(num_s / den_s))
    return (loss_risk, loss_surv)


def kernel(P_risk, P_surv, T, E):
    from concourse.bass_utils import run_bass_kernel_spmd

    nc = _get_nc()
    in_maps = make_in_maps(P_risk, P_surv, T, E)
    res = run_bass_kernel_spmd(nc, in_maps, core_ids=list(range(NCORES)))
    return combine_partials([res.results[c]["out_red"] for c in range(NCORES)])
